# revision 2
# baseline (speedup 1.0000x reference)
"""Trainium2 Bass kernel for the periodic flux-divergence stencil (v2, fp16).

    out = sum_ax  (v - roll(v, 1, ax)),  v = 0.5*(roll(M,-1,ax)+M)*(roll(mu,-1,ax)-mu)

over axes H, W of [B=16, 1, 1024, 1024] inputs, data-parallel over batch across
8 NeuronCores (2 images per core). Inputs are cast to fp16 on the host (tolerance
is 2e-2 rel; fp16 gives ~1e-3) halving HBM traffic and doubling DVE throughput.

Per-core plan, per 128-row block (9 blocks/image, 126 valid rows each):
  * W-direction: dw = mu_e - mu, aw = M_e + M, vw = 0.5*aw*dw on DVE
    (fp16 SBUF tensor_tensor runs at 2 elem/cycle/lane); the 1-column wrap
    fixups run on GpSimd (Pool) to keep them off the DVE critical path.
  * H-direction: [128x128] fp16 stencil matmuls on PE. The W-divergence
    (vw - vw_west) is folded into the PSUM accumulation as I*vw + (-I)*vw_w,
    so no DVE pass is spent on it.
  * PSUM evacuation on ScalarE with merged 2-bank [128,1024] copies.
"""
import sys

sys.path.insert(0, "/opt/trn_rl_repo")

import numpy as np

B, H, W = 16, 1024, 1024
N_CORES = 8
IMGS_PER_CORE = B // N_CORES          # 2
ROWS = IMGS_PER_CORE * H              # 2048
ADV = 126                             # valid rows per block
NBLK = (H + ADV - 1) // ADV           # 9
HALF = 512

_CACHE = {}


def _build(reps=1, fix="pool", parts="wh", align_test=False,
           shift_copies="none", evac="act", dma_split=False, io_bufs=3,
           pp=False):
    """fix: engine for the 1-column wrap fixups ("pool" or "dve").
    parts: subset of stages for engine-isolation benchmarking:
      "dma" = loads + store only; "w" = W-direction DVE work + store;
      "h" = H-direction PE/ACT/vh work + store; "wh" = full kernel.
    align_test: with parts="w", compute dw/aw with UNSHIFTED (4B-aligned)
    operands — wrong math, used to measure the DVE 2x-mode alignment penalty.
    """
    import contextlib

    import concourse.mybir as mybir
    from concourse import bacc
    from concourse.tile import TileContext

    f16 = mybir.dt.float16
    f32 = mybir.dt.float32
    Alu = mybir.AluOpType

    nc = bacc.Bacc(trn_type="TRN2", target_bir_lowering=False)

    M_d = nc.dram_tensor("m_in", [ROWS, W], f16, kind="ExternalInput")
    MU_d = nc.dram_tensor("mu_in", [ROWS, W], f16, kind="ExternalInput")
    ST_d = nc.dram_tensor("stencils", [128, 5 * 128], f16, kind="ExternalInput")
    OUT_d = nc.dram_tensor("out", [ROWS, W], f16, kind="ExternalOutput")

    with TileContext(nc) as tc:
        with (
            tc.tile_pool(name="consts", bufs=1) as cpool,
            tc.tile_pool(name="io", bufs=io_bufs) as iopool,
            tc.tile_pool(name="work", bufs=3 if pp else 2) as wpool,
            tc.tile_pool(name="hwork", bufs=4) as hpool,
            tc.tile_pool(name="psAD", bufs=3 if pp else 2, space="PSUM") as poolAD,
            tc.tile_pool(name="psC", bufs=2, space="PSUM") as poolC,
        ):
            st = cpool.tile([128, 5 * 128], f16)
            nc.sync.dma_start(out=st[:], in_=ST_d[:])
            lA = st[:, 0:128]      # (0.5*(I+U)).T
            lF = st[:, 128:256]    # (U-I).T
            lL = st[:, 256:384]    # (I-D).T
            lI = st[:, 384:512]    # I
            lN = st[:, 512:640]    # -I

            fixeng = nc.gpsimd if fix == "pool" else nc.vector

            if reps > 1:
                loop_ctx = tc.For_i(
                    0, reps, 1,
                    staggered_reset=True,
                    hint_engines=(
                        mybir.EngineType.PE, mybir.EngineType.DVE,
                        mybir.EngineType.Activation, mybir.EngineType.SP,
                        mybir.EngineType.Pool,
                    ),
                )
            else:
                loop_ctx = contextlib.nullcontext()
            with loop_ctx:
              for img in range(IMGS_PER_CORE):
                base = img * H
                for t in range(NBLK):
                    s = (ADV * t - 1) % H
                    n1 = min(128, H - s)
                    pieces = [(0, s, n1)]
                    if n1 < 128:
                        pieces.append((n1, 0, 128 - n1))

                    mu_t = iopool.tile([128, W], f16, tag="mu")
                    m_t = iopool.tile([128, W], f16, tag="m")
                    m_eng = nc.scalar if dma_split else nc.sync
                    for p0, r0, cnt in pieces:
                        nc.sync.dma_start(
                            out=mu_t[p0:p0 + cnt, :],
                            in_=MU_d[base + r0: base + r0 + cnt, :],
                        )
                        m_eng.dma_start(
                            out=m_t[p0:p0 + cnt, :],
                            in_=M_d[base + r0: base + r0 + cnt, :],
                        )

                    if parts == "dma":
                        out_t = wpool.tile([128, W], f16, tag="out")
                        r_out = ADV * t
                        nvalid = min(ADV, H - r_out)
                        s_eng = nc.scalar if (dma_split and t % 2) else nc.sync
                        s_eng.dma_start(
                            out=OUT_d[base + r_out: base + r_out + nvalid, :],
                            in_=mu_t[1:1 + nvalid, :],
                        )
                        continue

                    vw = None
                    if "w" in parts:
                        # ---- W-direction stencils ----
                        vw = wpool.tile([128, W + 1], f16, tag="vw")
                        if shift_copies != "none":
                            # build east-shifted copies on Pool/DVE so the
                            # main TTs have 4B-aligned operands (DVE 2x mode)
                            ceng = (nc.gpsimd if shift_copies == "pool"
                                    else nc.vector)
                            mu_s = wpool.tile([128, W], f16, tag="mus")
                            ceng.tensor_copy(
                                out=mu_s[:, 0:W - 1], in_=mu_t[:, 1:W])
                            ceng.tensor_copy(
                                out=mu_s[:, W - 1:W], in_=mu_t[:, 0:1])
                            m_s = wpool.tile([128, W], f16, tag="ms")
                            ceng.tensor_copy(
                                out=m_s[:, 0:W - 1], in_=m_t[:, 1:W])
                            ceng.tensor_copy(
                                out=m_s[:, W - 1:W], in_=m_t[:, 0:1])
                            dw = wpool.tile([128, W], f16, tag="dw")
                            nc.vector.tensor_tensor(
                                out=dw[:], in0=mu_s[:], in1=mu_t[:],
                                op=Alu.subtract)
                            aw = wpool.tile([128, W], f16, tag="aw")
                            nc.vector.tensor_tensor(
                                out=aw[:], in0=m_s[:], in1=m_t[:], op=Alu.add)
                        else:
                            sh = 0 if align_test else 1
                            dw = wpool.tile([128, W], f16, tag="dw")
                            nc.vector.tensor_tensor(
                                out=dw[:, 0:W - 1], in0=mu_t[:, sh:sh + W - 1],
                                in1=mu_t[:, 0:W - 1], op=Alu.subtract,
                            )
                            fixeng.tensor_tensor(
                                out=dw[:, W - 1:W], in0=mu_t[:, 0:1],
                                in1=mu_t[:, W - 1:W], op=Alu.subtract,
                            )
                            aw = wpool.tile([128, W], f16, tag="aw")
                            nc.vector.tensor_tensor(
                                out=aw[:, 0:W - 1], in0=m_t[:, sh:sh + W - 1],
                                in1=m_t[:, 0:W - 1], op=Alu.add,
                            )
                            fixeng.tensor_tensor(
                                out=aw[:, W - 1:W], in0=m_t[:, 0:1],
                                in1=m_t[:, W - 1:W], op=Alu.add,
                            )
                        # vw at columns 1..W; column 0 = wrap copy of col W
                        nc.vector.scalar_tensor_tensor(
                            out=vw[:, 1:W + 1], in0=aw[:], scalar=0.5, in1=dw[:],
                            op0=Alu.mult, op1=Alu.mult,
                        )
                        fixeng.tensor_copy(out=vw[:, 0:1], in_=vw[:, W:W + 1])

                    if parts == "w":
                        r_out = ADV * t
                        nvalid = min(ADV, H - r_out)
                        nc.sync.dma_start(
                            out=OUT_d[base + r_out: base + r_out + nvalid, :],
                            in_=vw[1:1 + nvalid, 1:W + 1],
                        )
                        continue

                    # ---- H-direction (partition stencils on PE) ----
                    # batch same-weight matmuls to amortize LDWEIGHTS
                    psAD = [poolAD.tile([128, 2 * HALF], f32, tag="ad",
                                        name=f"psAD{h}")
                            for h in range(2)]
                    for h in range(2):
                        nc.tensor.matmul(
                            psAD[h][:, 0:HALF], lA,
                            m_t[:, h * HALF:(h + 1) * HALF],
                            start=True, stop=True,
                        )
                    for h in range(2):
                        nc.tensor.matmul(
                            psAD[h][:, HALF:2 * HALF], lF,
                            mu_t[:, h * HALF:(h + 1) * HALF],
                            start=True, stop=True,
                        )
                    vhs = []
                    for h in range(2):
                        sAD = hpool.tile([128, 2 * HALF], f16, tag="sad")
                        if evac == "dve_sad":
                            nc.vector.tensor_copy(out=sAD[:], in_=psAD[h][:])
                        else:
                            nc.scalar.copy(out=sAD[:], in_=psAD[h][:])
                        vh = hpool.tile([128, HALF], f16, tag="vh")
                        nc.vector.tensor_tensor(
                            out=vh[:], in0=sAD[:, 0:HALF],
                            in1=sAD[:, HALF:2 * HALF], op=Alu.mult,
                        )
                        vhs.append(vh)

                    w_on = vw is not None
                    if pp:
                        psCs = [poolC.tile([128, HALF], f32, tag="c",
                                           name=f"psC{h}") for h in range(2)]
                        csl = [psCs[0][:], psCs[1][:]]
                    else:
                        psC = poolC.tile([128, 2 * HALF], f32, tag="c")
                        csl = [psC[:, 0:HALF], psC[:, HALF:2 * HALF]]
                    for h in range(2):
                        nc.tensor.matmul(
                            csl[h], lL, vhs[h], start=True, stop=not w_on,
                        )
                    if w_on:
                        for h in range(2):
                            nc.tensor.matmul(
                                csl[h], lI,
                                vw[:, 1 + h * HALF:1 + (h + 1) * HALF],
                                start=False, stop=False,
                            )
                        for h in range(2):
                            nc.tensor.matmul(
                                csl[h], lN,
                                vw[:, h * HALF:(h + 1) * HALF],
                                start=False, stop=True,
                            )

                    out_t = wpool.tile([128, W], f16, tag="out")
                    if evac == "act" and not pp:
                        nc.scalar.copy(out=out_t[:], in_=psC[:])
                    elif evac == "split" or pp:
                        nc.scalar.copy(out=out_t[:, 0:HALF], in_=csl[0])
                        nc.vector.tensor_copy(out=out_t[:, HALF:W], in_=csl[1])
                    else:  # "dve_out"
                        nc.vector.tensor_copy(out=out_t[:], in_=csl[0])

                    r_out = ADV * t
                    nvalid = min(ADV, H - r_out)
                    s_eng = nc.scalar if (dma_split and t % 2) else nc.sync
                    s_eng.dma_start(
                        out=OUT_d[base + r_out: base + r_out + nvalid, :],
                        in_=out_t[1:1 + nvalid, :],
                    )

    nc.compile()
    return nc


def _stencils():
    A = np.zeros((128, 128), dtype=np.float32)
    F = np.zeros((128, 128), dtype=np.float32)
    L = np.zeros((128, 128), dtype=np.float32)
    for r in range(127):
        A[r, r] = 0.5
        A[r, r + 1] = 0.5
        F[r, r] = -1.0
        F[r, r + 1] = 1.0
    A[127, 127] = 0.5
    F[127, 127] = -1.0
    for r in range(1, 128):
        L[r, r] = 1.0
        L[r, r - 1] = -1.0
    I = np.eye(128, dtype=np.float32)
    st = np.concatenate([A.T, F.T, L.T, I, -I], axis=1)
    return np.ascontiguousarray(st).astype(np.float16)


def make_in_maps(M, mu):
    M = np.asarray(M, dtype=np.float32).reshape(B, H, W).astype(np.float16)
    mu = np.asarray(mu, dtype=np.float32).reshape(B, H, W).astype(np.float16)
    st = _stencils()
    in_maps = []
    for c in range(N_CORES):
        i0 = c * IMGS_PER_CORE
        in_maps.append({
            "m_in": np.ascontiguousarray(M[i0:i0 + IMGS_PER_CORE].reshape(ROWS, W)),
            "mu_in": np.ascontiguousarray(mu[i0:i0 + IMGS_PER_CORE].reshape(ROWS, W)),
            "stencils": st,
        })
    return in_maps


BEST = dict(parts="wh")


def kernel(M, mu):
    from concourse.bass_utils import run_bass_kernel_spmd

    if "nc" not in _CACHE:
        _CACHE["nc"] = _build(**BEST)
    nc = _CACHE["nc"]

    in_maps = make_in_maps(M, mu)
    res = run_bass_kernel_spmd(nc, in_maps, core_ids=list(range(N_CORES)))
    out = np.empty((B, H, W), dtype=np.float32)
    for c in range(N_CORES):
        out[c * IMGS_PER_CORE:(c + 1) * IMGS_PER_CORE] = (
            res.results[c]["out"].astype(np.float32).reshape(IMGS_PER_CORE, H, W)
        )
    return out.reshape(B, 1, H, W)
